# revision 1
# baseline (speedup 1.0000x reference)
"""Cross-attention kernel for Trainium2 (8 NeuronCores, batch-parallel).

Math per batch b (reference semantics):
  q = queries[b].reshape(C, N).T + q_pos        # [N, C]
  k = keys[b].reshape(C, N).T + k_pos
  v = values[b].reshape(C, N).T                 # [N, C]
  out = softmax(q @ k.T / 16) @ v, returned as [C, N] (c-major)

Device layout (per core = one batch):
  All matmuls in f32r (TF32 mode, 1 PE cycle/row).  S is computed transposed
  (S^T[k, q]) so that exp(S^T) tiles are directly the rhs of the O matmul
  (O^T = V^T A^T) and the softmax denominator comes from a ones-column
  matmul -- no on-chip transposes anywhere.
"""

import numpy as np

import concourse.bass as bass
import concourse.tile as tile
import concourse.mybir as mybir
from concourse import bacc
from concourse.bass_utils import run_bass_kernel_spmd

P = 128          # partitions
C = 256          # qk/v channel dim
N = 4096         # sequence (64*64)
B = 8            # batch == n_cores
QW = 512         # query block width (max fp32-class matmul free dim)
NQB = N // QW    # 8 query blocks
NKO = N // P     # 32 key chunks
KPB = QW // P    # key chunks per K block tile
SCALE = 1.0 / 16.0  # 1/sqrt(C)

F32 = mybir.dt.float32
F32R = mybir.dt.float32r
AF = mybir.ActivationFunctionType

_NC_CACHE = None


def tf32_round(x: np.ndarray) -> np.ndarray:
    u = x.view(np.uint32)
    u = (u + np.uint32(0x1000)) & np.uint32(0xFFFFE000)
    return u.view(np.float32)


def build_nc(atp_bufs=6, raw_bufs=3, ps_s_bufs=4, ps_o_bufs=1, lag=3):
    nc = bacc.Bacc(None, target_bir_lowering=False)
    qt = nc.dram_tensor("qt", [C, N], F32, kind="ExternalInput")
    kt = nc.dram_tensor("kt", [C, N], F32, kind="ExternalInput")
    v = nc.dram_tensor("v", [N, C], F32R, kind="ExternalInput")
    qp = nc.dram_tensor("qp", [C, N], F32, kind="ExternalInput")
    kp = nc.dram_tensor("kp", [C, N], F32, kind="ExternalInput")
    o = nc.dram_tensor("o", [C, N], F32, kind="ExternalOutput")

    qt3 = qt.rearrange("(co p) n -> p co n", p=P)
    kt3 = kt.rearrange("(co p) n -> p co n", p=P)
    qp3 = qp.rearrange("(co p) n -> p co n", p=P)
    kp3 = kp.rearrange("(co p) n -> p co n", p=P)
    v3 = v.rearrange("(ko p) c -> p ko c", p=P)

    with tile.TileContext(nc) as tc:
        with (
            tc.tile_pool(name="consts", bufs=1) as consts,
            tc.tile_pool(name="qk", bufs=NQB) as qk,
            tc.tile_pool(name="vp", bufs=NKO) as vp,
            tc.tile_pool(name="raw", bufs=raw_bufs) as raw,
            tc.tile_pool(name="atp", bufs=atp_bufs) as atp,
            tc.tile_pool(name="small", bufs=2) as small,
            tc.tile_pool(name="outp", bufs=2) as outp,
            tc.tile_pool(name="ps_s", bufs=ps_s_bufs, space="PSUM") as ps_s,
            tc.tile_pool(name="ps_o", bufs=ps_o_bufs, space="PSUM") as ps_o,
            tc.tile_pool(name="ps_r", bufs=1, space="PSUM") as ps_r,
            tc.tile_pool(name="ps_b", bufs=1, space="PSUM") as ps_b,
        ):
            ones_f = consts.tile([P, 2], F32, tag="ones_f")
            nc.vector.memset(ones_f, 1.0)
            ones_c = consts.tile([P, 2], F32R, tag="ones_c")
            nc.vector.tensor_copy(ones_c, ones_f)
            ones_rf = consts.tile([1, P], F32, tag="ones_rf")
            nc.vector.memset(ones_rf, 1.0)
            ones_r = consts.tile([1, P], F32R, tag="ones_r")
            nc.vector.tensor_copy(ones_r, ones_rf)

            # K blocks (pos-added, f32r) and V chunks, emitted in deadline
            # order: block 0's dependencies first (K0, V0..3), then K(jb)
            # interleaved with the V chunks needed just before it.
            def load_kblk(j):
                sl = slice(j * QW, (j + 1) * QW)
                kraw = raw.tile([P, 2, QW], F32, tag="kraw")
                kpos = raw.tile([P, 2, QW], F32, tag="kpos")
                nc.sync.dma_start(kraw, kt3[:, :, sl])
                nc.sync.dma_start(kpos, kp3[:, :, sl])
                kb = qk.tile([P, 2, QW], F32R, tag="kblk")
                if j == 0:
                    for co in range(2):
                        nc.vector.tensor_add(kb[:, co, :], kraw[:, co, :],
                                             kpos[:, co, :])
                else:
                    nc.vector.tensor_add(kb, kraw, kpos)
                return kb

            def load_vchunk(ko):
                vc = vp.tile([P, C], F32R, tag="v")
                nc.sync.dma_start(vc, v3[:, ko, :])
                return vc

            kblks = {}
            vcs = {}
            kblks[0] = load_kblk(0)

            def emit_epilogue(j, po0, po1, pr):
                sl = slice(j * QW, (j + 1) * QW)
                inv = small.tile([1, QW], F32R, tag="inv")
                with nc.allow_low_precision(
                    reason="TF32 rounding of softmax reciprocal"
                ):
                    nc.vector.reciprocal(inv, pr[0:1, :])
                pb = ps_b.tile([P, QW], F32, tag="b")
                nc.tensor.matmul(pb, ones_r, inv, start=True, stop=True)
                bs = small.tile([P, QW], F32, tag="bs")
                nc.vector.tensor_copy(bs, pb)
                oo0 = outp.tile([P, QW], F32, tag="oo0")
                nc.vector.tensor_mul(oo0, po0, bs)
                nc.sync.dma_start(o[0:P, sl], oo0)
                oo1 = outp.tile([P, QW], F32, tag="oo1")
                nc.vector.tensor_mul(oo1, po1, bs)
                nc.sync.dma_start(o[P:C, sl], oo1)

            pending = None
            for j in range(NQB):
                sl = slice(j * QW, (j + 1) * QW)
                qraw = raw.tile([P, 2, QW], F32, tag="qraw")
                qpos = raw.tile([P, 2, QW], F32, tag="qpos")
                nc.sync.dma_start(qraw, qt3[:, :, sl])
                nc.sync.dma_start(qpos, qp3[:, :, sl])
                qb = qk.tile([P, 2, QW], F32R, tag="qblk")
                if j == 0:
                    for co in range(2):
                        nc.vector.tensor_add(qb[:, co, :], qraw[:, co, :],
                                             qpos[:, co, :])
                else:
                    nc.vector.tensor_add(qb, qraw, qpos)

                if j == 0:
                    # deadline-ordered remaining loads: V(4jb..) then K(jb+1)
                    for jb in range(NQB):
                        for ko in range(4 * jb, 4 * jb + 4):
                            vcs[ko] = load_vchunk(ko)
                        if jb + 1 < NQB:
                            kblks[jb + 1] = load_kblk(jb + 1)

                po0 = ps_o.tile([P, QW], F32, tag="o0")
                po1 = ps_o.tile([P, QW], F32, tag="o1")
                pr = ps_r.tile([2, QW], F32, tag="r")

                a_q = {}

                for ko in range(NKO):
                    pss = ps_s.tile([P, QW], F32, tag="s")
                    jb, koff = divmod(ko, KPB)
                    for co in range(2):
                        nc.tensor.matmul(
                            pss,
                            kblks[jb][:, co, koff * P : (koff + 1) * P],
                            qb[:, co, :],
                            start=(co == 0),
                            stop=(co == 1),
                        )
                    a = atp.tile([P, QW], F32R, tag="a")
                    nc.scalar.activation(a, pss, AF.Exp, scale=SCALE)
                    a_q[ko] = a

                    if ko >= lag:
                        pko = ko - lag
                        av = a_q[pko]
                        nc.tensor.matmul(po0, vcs[pko][:, 0:P], av,
                                         start=(pko == 0), stop=False)
                        nc.tensor.matmul(po1, vcs[pko][:, P:C], av,
                                         start=(pko == 0), stop=False)
                        nc.tensor.matmul(pr, ones_c, av,
                                         start=(pko == 0), stop=False)
                        del a_q[pko]

                    if ko == 2 and pending is not None:
                        emit_epilogue(*pending)
                        pending = None

                # drain remaining lagged chunks; last closes the groups
                for pko in range(NKO - lag, NKO):
                    av = a_q[pko]
                    last = pko == NKO - 1
                    nc.tensor.matmul(po0, vcs[pko][:, 0:P], av,
                                     start=False, stop=last)
                    nc.tensor.matmul(po1, vcs[pko][:, P:C], av,
                                     start=False, stop=last)
                    nc.tensor.matmul(pr, ones_c, av, start=False, stop=last)
                    del a_q[pko]
                pending = (j, po0, po1, pr)

            emit_epilogue(*pending)

    nc.compile()
    return nc


def _get_nc():
    global _NC_CACHE
    if _NC_CACHE is None:
        _NC_CACHE = build_nc()
    return _NC_CACHE


def make_in_maps(queries, keys, values, q_pos_embedding, k_pos_embedding):
    queries = np.asarray(queries, dtype=np.float32)
    keys = np.asarray(keys, dtype=np.float32)
    values = np.asarray(values, dtype=np.float32)
    qpT = np.ascontiguousarray(
        np.asarray(q_pos_embedding, dtype=np.float32).reshape(N, C).T
    )
    kpT = np.ascontiguousarray(
        np.asarray(k_pos_embedding, dtype=np.float32).reshape(N, C).T
    )
    in_maps = []
    for b in range(B):
        vT = tf32_round(
            np.ascontiguousarray(values[b].reshape(C, N).T)
        )
        in_maps.append({
            "qt": np.ascontiguousarray(queries[b].reshape(C, N)),
            "kt": np.ascontiguousarray(keys[b].reshape(C, N)),
            "v": vT,
            "qp": qpT,
            "kp": kpT,
        })
    return in_maps


def kernel(queries, keys, values, q_pos_embedding, k_pos_embedding):
    nc = _get_nc()
    in_maps = make_in_maps(queries, keys, values, q_pos_embedding,
                           k_pos_embedding)
    res = run_bass_kernel_spmd(nc, in_maps, core_ids=list(range(B)))
    out = np.stack([r["o"].reshape(C, 64, 64) for r in res.results])
    return out.astype(np.float32)


def build_nc_trivial():
    """Same I/O signature, minimal work: used by test.py to subtract the
    per-call transfer/dispatch overhead from wall-clock timing."""
    nc = bacc.Bacc(None, target_bir_lowering=False)
    qt = nc.dram_tensor("qt", [C, N], F32, kind="ExternalInput")
    kt = nc.dram_tensor("kt", [C, N], F32, kind="ExternalInput")
    v = nc.dram_tensor("v", [N, C], F32R, kind="ExternalInput")
    qp = nc.dram_tensor("qp", [C, N], F32, kind="ExternalInput")
    kp = nc.dram_tensor("kp", [C, N], F32, kind="ExternalInput")
    o = nc.dram_tensor("o", [C, N], F32, kind="ExternalOutput")
    with tile.TileContext(nc) as tc:
        with tc.tile_pool(name="sb", bufs=2) as sb:
            t = sb.tile([P, 2, N], F32, tag="t")
            nc.sync.dma_start(t, qt.rearrange("(co p) n -> p co n", p=P))
            nc.sync.dma_start(o.rearrange("(co p) n -> p co n", p=P), t)
    nc.compile()
    return nc



# revision 3
# speedup vs baseline: 1.3378x; 1.3378x over previous
"""Cross-attention kernel for Trainium2 (8 NeuronCores, batch-parallel).

Math per batch b (reference semantics):
  q = queries[b].reshape(C, N).T + q_pos        # [N, C]
  k = keys[b].reshape(C, N).T + k_pos
  v = values[b].reshape(C, N).T                 # [N, C]
  out = softmax(q @ k.T / 16) @ v               # [N, Cv]

Device layout (per core = one batch):
  S is computed transposed (S^T[k, q]) so exp(S^T) tiles are directly the
  STATIONARY operand of the O matmul (O[q, c] = sum_k A^T[k, q]^T V[k, c]).
  V chunks are augmented with a ones column so the softmax denominator
  accumulates in PSUM column C for free; the final normalization is a
  per-partition scaled copy on the (otherwise idle) scalar engine.

  Q/K have the position embeddings folded in host-side and are split into
  fp8e4m3 hi+lo halves; S = Kh Qh + Kh Ql + Kl Qh runs as 256-deep
  DoubleRow fp8 matmuls (0.5 PE cycles/row -- 2x the f32r rate).  The
  dropped Kl Ql term and residual quantization contribute ~1e-3 relative
  error on the logits.  The O matmul stays f32r (A stationary, V moving).
"""

import numpy as np

import concourse.bass as bass
import concourse.tile as tile
import concourse.mybir as mybir
from concourse import bacc
from concourse.bass_utils import run_bass_kernel_spmd

P = 128          # partitions
C = 256          # qk/v channel dim
N = 4096         # sequence (64*64)
B = 8            # batch == n_cores
QW = 512         # query block width (max matmul moving free dim)
NQB = N // QW    # 8 query blocks
NKO = N // P     # 32 key chunks
KPB = QW // P    # key chunks per K block tile
LAG = 6          # O-matmul lag behind exp, in key chunks
SCALE = 1.0 / 16.0  # 1/sqrt(C)

F32 = mybir.dt.float32
F32R = mybir.dt.float32r
F8 = mybir.dt.float8e4
AF = mybir.ActivationFunctionType
DR = mybir.MatmulPerfMode.DoubleRow

_NC_CACHE = None


def tf32_round(x: np.ndarray) -> np.ndarray:
    u = x.view(np.uint32)
    u = (u + np.uint32(0x1000)) & np.uint32(0xFFFFE000)
    return u.view(np.float32)


def build_nc(ps_s_bufs=3, po_bufs=5, atp_bufs=LAG + 4):
    nc = bacc.Bacc(None, target_bir_lowering=False)
    qh = nc.dram_tensor("qh", [C, N], F8, kind="ExternalInput")
    ql = nc.dram_tensor("ql", [C, N], F8, kind="ExternalInput")
    kh = nc.dram_tensor("kh", [C, N], F8, kind="ExternalInput")
    kl = nc.dram_tensor("kl", [C, N], F8, kind="ExternalInput")
    v = nc.dram_tensor("v", [N, C], F32R, kind="ExternalInput")
    o = nc.dram_tensor("o", [N, C], F32, kind="ExternalOutput")

    qh3 = qh.rearrange("(co p) n -> p co n", p=P)
    ql3 = ql.rearrange("(co p) n -> p co n", p=P)
    kh3 = kh.rearrange("(co p) n -> p co n", p=P)
    kl3 = kl.rearrange("(co p) n -> p co n", p=P)
    v3 = v.rearrange("(ko p) c -> p ko c", p=P)
    o3 = o.rearrange("(nb p) c -> p nb c", p=P)

    with tile.TileContext(nc) as tc:
        with (
            tc.tile_pool(name="consts", bufs=1) as consts,
            tc.tile_pool(name="kk", bufs=NQB) as kk,
            tc.tile_pool(name="qq", bufs=2) as qq,
            tc.tile_pool(name="vp", bufs=NKO) as vp,
            tc.tile_pool(name="atp", bufs=atp_bufs) as atp,
            tc.tile_pool(name="small", bufs=8) as small,
            tc.tile_pool(name="outp", bufs=4) as outp,
            tc.tile_pool(name="ps_s", bufs=ps_s_bufs, space="PSUM") as ps_s,
            tc.tile_pool(name="ps_o", bufs=po_bufs, space="PSUM") as ps_o,
        ):
            ones_f = consts.tile([P, 1], F32, tag="ones_f")
            nc.vector.memset(ones_f, 1.0)

            def load_kblk(j):
                sl = slice(j * QW, (j + 1) * QW)
                kb_h = kk.tile([P, 2, QW], F8, tag="kh")
                kb_l = kk.tile([P, 2, QW], F8, tag="kl")
                nc.sync.dma_start(kb_h, kh3[:, :, sl])
                nc.sync.dma_start(kb_l, kl3[:, :, sl])
                return kb_h, kb_l

            def load_vchunk(ko):
                vc = vp.tile([P, C + 1], F32R, tag="v")
                nc.sync.dma_start(vc[:, 0:C], v3[:, ko, :])
                nc.vector.tensor_copy(vc[:, C : C + 1], ones_f)
                return vc

            kblks = {}
            vcs = {}
            kblks[0] = load_kblk(0)

            def emit_epilogue(j, pos):
                for qs in range(4):
                    inv = small.tile([P, 1], F32, tag="inv")
                    nc.vector.reciprocal(inv, pos[qs][:, C : C + 1])
                    ot = outp.tile([P, C], F32, tag="ot")
                    nc.scalar.activation(ot, pos[qs][:, 0:C], AF.Copy,
                                         scale=inv)
                    nc.sync.dma_start(o3[:, 4 * j + qs, :], ot)

            pending = None
            for j in range(NQB):
                sl = slice(j * QW, (j + 1) * QW)
                qb_h = qq.tile([P, 2, QW], F8, tag="qh")
                qb_l = qq.tile([P, 2, QW], F8, tag="ql")
                nc.sync.dma_start(qb_h, qh3[:, :, sl])
                nc.sync.dma_start(qb_l, ql3[:, :, sl])

                if j == 0:
                    # deadline-ordered remaining loads: V(4jb..) then K(jb+1)
                    for jb in range(NQB):
                        for ko in range(4 * jb, 4 * jb + 4):
                            vcs[ko] = load_vchunk(ko)
                        if jb + 1 < NQB:
                            kblks[jb + 1] = load_kblk(jb + 1)

                if pending is not None:
                    emit_epilogue(*pending)
                    pending = None

                po = [ps_o.tile([P, C + 1], F32, tag="po", name=f"po{qs}",
                                padded_shape=[P, QW]) for qs in range(4)]

                a_q = {}

                def o_matmuls(ko):
                    av = a_q[ko]
                    for qs in range(4):
                        nc.tensor.matmul(
                            po[qs],
                            av[:, qs * P : (qs + 1) * P],
                            vcs[ko],
                            start=(ko == 0),
                            stop=(ko == NKO - 1),
                        )
                    del a_q[ko]

                for ko in range(NKO):
                    pss = ps_s.tile([P, QW], F32, tag="s")
                    jb, koff = divmod(ko, KPB)
                    ksl = slice(koff * P, (koff + 1) * P)
                    kb_h, kb_l = kblks[jb]
                    nc.tensor.matmul(pss, kb_h[:, :, ksl], qb_h,
                                     start=True, stop=False, perf_mode=DR)
                    nc.tensor.matmul(pss, kb_h[:, :, ksl], qb_l,
                                     start=False, stop=False, perf_mode=DR)
                    nc.tensor.matmul(pss, kb_l[:, :, ksl], qb_h,
                                     start=False, stop=True, perf_mode=DR)
                    a = atp.tile([P, QW], F32R, tag="a")
                    nc.scalar.activation(a, pss, AF.Exp, scale=SCALE)
                    a_q[ko] = a

                    if ko >= LAG:
                        o_matmuls(ko - LAG)

                for ko in range(NKO - LAG, NKO):
                    o_matmuls(ko)
                pending = (j, po)

            emit_epilogue(*pending)

    nc.compile()
    return nc


def _get_nc():
    global _NC_CACHE
    if _NC_CACHE is None:
        _NC_CACHE = build_nc()
    return _NC_CACHE


def make_in_maps(queries, keys, values, q_pos_embedding, k_pos_embedding):
    queries = np.asarray(queries, dtype=np.float32)
    keys = np.asarray(keys, dtype=np.float32)
    values = np.asarray(values, dtype=np.float32)
    fp8 = mybir.dt.np(F8)
    qpT = np.asarray(q_pos_embedding, dtype=np.float32).reshape(N, C).T
    kpT = np.asarray(k_pos_embedding, dtype=np.float32).reshape(N, C).T
    in_maps = []
    for b in range(B):
        qt = queries[b].reshape(C, N) + qpT
        kt = keys[b].reshape(C, N) + kpT
        qh8 = qt.astype(fp8)
        ql8 = (qt - qh8.astype(np.float32)).astype(fp8)
        kh8 = kt.astype(fp8)
        kl8 = (kt - kh8.astype(np.float32)).astype(fp8)
        vT = tf32_round(np.ascontiguousarray(values[b].reshape(C, N).T))
        in_maps.append({
            "qh": np.ascontiguousarray(qh8),
            "ql": np.ascontiguousarray(ql8),
            "kh": np.ascontiguousarray(kh8),
            "kl": np.ascontiguousarray(kl8),
            "v": vT,
        })
    return in_maps


def kernel(queries, keys, values, q_pos_embedding, k_pos_embedding):
    nc = _get_nc()
    in_maps = make_in_maps(queries, keys, values, q_pos_embedding,
                           k_pos_embedding)
    res = run_bass_kernel_spmd(nc, in_maps, core_ids=list(range(B)))
    out = np.stack([r["o"].T.reshape(C, 64, 64) for r in res.results])
    return out.astype(np.float32)


# revision 4
# speedup vs baseline: 1.4297x; 1.0687x over previous
"""Cross-attention kernel for Trainium2 (8 NeuronCores, batch-parallel).

Math per batch b (reference semantics):
  q = queries[b].reshape(C, N).T + q_pos        # [N, C]
  k = keys[b].reshape(C, N).T + k_pos
  v = values[b].reshape(C, N).T                 # [N, C]
  out = softmax(q @ k.T / 16) @ v               # [N, Cv]

Device layout (per core = one batch):
  S is computed transposed (S^T[k, q]) so exp(S^T) tiles are directly the
  STATIONARY operand of the O matmul (O[q, c] = sum_k A^T[k, q]^T V[k, c]).
  V chunks are augmented with two ones columns (f32r matmuls need an even
  moving free dim) so the softmax denominator accumulates in PSUM columns
  C/C+1 for free; the final normalization is a per-partition reciprocal +
  scalar multiply on the (otherwise idle) vector engine, staggered into the
  next block's key loop so it never clogs the activation queue.

  Q/K have the position embeddings folded in host-side and are split into
  fp8e4m3 hi+lo halves; S = Kh Qh + Kh Ql + Kl Qh runs as 256-deep
  DoubleRow fp8 matmuls (0.5 PE cycles/row -- 2x the f32r rate).  The
  dropped Kl Ql term and residual quantization contribute ~1e-3 relative
  error on the logits.  The O matmul stays f32r (A stationary, V moving).
"""

import numpy as np

import concourse.bass as bass
import concourse.tile as tile
import concourse.mybir as mybir
from concourse import bacc
from concourse.bass_utils import run_bass_kernel_spmd

P = 128          # partitions
C = 256          # qk/v channel dim
CA = C + 2       # v width augmented with ones columns (must be even)
N = 4096         # sequence (64*64)
B = 8            # batch == n_cores
QW = 512         # query block width (max matmul moving free dim)
NQB = N // QW    # 8 query blocks
NKO = N // P     # 32 key chunks
KPB = QW // P    # key chunks per K block tile
VB = 4           # v chunks loaded per DMA
LAG = 6          # O-matmul lag behind exp, in key chunks
SCALE = 1.0 / 16.0  # 1/sqrt(C)

F32 = mybir.dt.float32
F32R = mybir.dt.float32r
F8 = mybir.dt.float8e4
AF = mybir.ActivationFunctionType
DR = mybir.MatmulPerfMode.DoubleRow

_NC_CACHE = None


def tf32_round(x: np.ndarray) -> np.ndarray:
    u = x.view(np.uint32)
    u = (u + np.uint32(0x1000)) & np.uint32(0xFFFFE000)
    return u.view(np.float32)


def build_nc(ps_s_bufs=4, po_bufs=4, atp_bufs=LAG + 4):
    nc = bacc.Bacc(None, target_bir_lowering=False)
    q8 = nc.dram_tensor("q8", [2, C, N], F8, kind="ExternalInput")
    k8 = nc.dram_tensor("k8", [2, C, N], F8, kind="ExternalInput")
    v = nc.dram_tensor("v", [N, C], F32R, kind="ExternalInput")
    o = nc.dram_tensor("o", [N, C], F32, kind="ExternalOutput")

    q84 = q8.rearrange("hl (co p) n -> p hl co n", p=P)
    k84 = k8.rearrange("hl (co p) n -> p hl co n", p=P)
    v3 = v.rearrange("(g p) c -> p g c", p=P)
    o3 = o.rearrange("(nb p) c -> p nb c", p=P)

    with tile.TileContext(nc) as tc:
        with (
            tc.tile_pool(name="consts", bufs=1) as consts,
            tc.tile_pool(name="kk", bufs=NQB) as kk,
            tc.tile_pool(name="qq", bufs=2) as qq,
            tc.tile_pool(name="vp", bufs=NKO // VB) as vp,
            tc.tile_pool(name="atp", bufs=atp_bufs) as atp,
            tc.tile_pool(name="small", bufs=8) as small,
            tc.tile_pool(name="outp", bufs=4) as outp,
            tc.tile_pool(name="ps_s", bufs=ps_s_bufs, space="PSUM") as ps_s,
            tc.tile_pool(name="ps_o", bufs=po_bufs, space="PSUM") as ps_o,
        ):
            ones_f = consts.tile([P, VB, CA - C], F32, tag="ones_f")
            nc.vector.memset(ones_f, 1.0)

            def load_kblk(j):
                sl = slice(j * QW, (j + 1) * QW)
                kb = kk.tile([P, 2, 2, QW], F8, tag="k8")
                nc.sync.dma_start(kb, k84[:, :, :, sl])
                return kb

            def load_vgroup(g):
                vg = vp.tile([P, VB, CA], F32R, tag="v")
                nc.sync.dma_start(vg[:, :, 0:C], v3[:, g * VB : (g + 1) * VB, :])
                nc.vector.tensor_copy(vg[:, :, C:CA], ones_f)
                return vg

            kblks = {}
            vgs = {}
            kblks[0] = load_kblk(0)

            def epilogue_piece(j, po, qs, on_act=False):
                inv = small.tile([P, 1], F32, tag="inv")
                nc.vector.reciprocal(inv, po[qs][:, C : C + 1])
                ot = outp.tile([P, C], F32, tag="ot")
                if on_act:
                    nc.scalar.activation(ot, po[qs][:, 0:C], AF.Copy,
                                         scale=inv)
                else:
                    nc.vector.tensor_scalar_mul(ot, po[qs][:, 0:C], inv)
                nc.sync.dma_start(o3[:, 4 * j + qs, :], ot)

            pending = None
            for j in range(NQB):
                sl = slice(j * QW, (j + 1) * QW)
                qb = qq.tile([P, 2, 2, QW], F8, tag="q8")
                nc.sync.dma_start(qb, q84[:, :, :, sl])

                if j == 0:
                    # deadline-ordered remaining loads: K(jb+1) then V(jb)
                    for jb in range(NQB):
                        if jb + 1 < NQB:
                            kblks[jb + 1] = load_kblk(jb + 1)
                        vgs[jb] = load_vgroup(jb)

                po = [ps_o.tile([P, CA], F32, tag="po", name=f"po{qs}",
                                padded_shape=[P, QW]) for qs in range(4)]

                a_q = {}

                def o_matmuls(ko):
                    av = a_q[ko]
                    vc = vgs[ko // VB][:, ko % VB, :]
                    for qs in range(4):
                        nc.tensor.matmul(
                            po[qs],
                            av[:, qs * P : (qs + 1) * P],
                            vc,
                            start=(ko == 0),
                            stop=(ko == NKO - 1),
                        )
                    del a_q[ko]

                for ko in range(NKO):
                    pss = ps_s.tile([P, QW], F32, tag="s")
                    jb, koff = divmod(ko, KPB)
                    ksl = slice(koff * P, (koff + 1) * P)
                    kb = kblks[jb]
                    nc.tensor.matmul(pss, kb[:, 0, :, ksl], qb[:, 0, :, :],
                                     start=True, stop=False, perf_mode=DR)
                    nc.tensor.matmul(pss, kb[:, 0, :, ksl], qb[:, 1, :, :],
                                     start=False, stop=False, perf_mode=DR)
                    nc.tensor.matmul(pss, kb[:, 1, :, ksl], qb[:, 0, :, :],
                                     start=False, stop=True, perf_mode=DR)
                    a = atp.tile([P, QW], F32R, tag="a")
                    nc.scalar.activation(a, pss, AF.Exp, scale=SCALE)
                    a_q[ko] = a

                    if pending is not None and ko < 4:
                        epilogue_piece(pending[0], pending[1], ko)
                        if ko == 3:
                            pending = None

                    if ko >= LAG:
                        o_matmuls(ko - LAG)

                for ko in range(NKO - LAG, NKO):
                    o_matmuls(ko)
                pending = (j, po)

            # final block: split the epilogue across DVE and ACT
            for qs in range(4):
                epilogue_piece(pending[0], pending[1], qs, on_act=(qs % 2 == 1))

    nc.compile()
    return nc


def _get_nc():
    global _NC_CACHE
    if _NC_CACHE is None:
        _NC_CACHE = build_nc()
    return _NC_CACHE


def make_in_maps(queries, keys, values, q_pos_embedding, k_pos_embedding):
    queries = np.asarray(queries, dtype=np.float32)
    keys = np.asarray(keys, dtype=np.float32)
    values = np.asarray(values, dtype=np.float32)
    fp8 = mybir.dt.np(F8)
    qpT = np.asarray(q_pos_embedding, dtype=np.float32).reshape(N, C).T
    kpT = np.asarray(k_pos_embedding, dtype=np.float32).reshape(N, C).T
    in_maps = []
    for b in range(B):
        qt = queries[b].reshape(C, N) + qpT
        kt = keys[b].reshape(C, N) + kpT
        qh8 = qt.astype(fp8)
        ql8 = (qt - qh8.astype(np.float32)).astype(fp8)
        kh8 = kt.astype(fp8)
        kl8 = (kt - kh8.astype(np.float32)).astype(fp8)
        vT = tf32_round(np.ascontiguousarray(values[b].reshape(C, N).T))
        in_maps.append({
            "q8": np.ascontiguousarray(np.stack([qh8, ql8])),
            "k8": np.ascontiguousarray(np.stack([kh8, kl8])),
            "v": vT,
        })
    return in_maps


def kernel(queries, keys, values, q_pos_embedding, k_pos_embedding):
    nc = _get_nc()
    in_maps = make_in_maps(queries, keys, values, q_pos_embedding,
                           k_pos_embedding)
    res = run_bass_kernel_spmd(nc, in_maps, core_ids=list(range(B)))
    out = np.stack([r["o"].T.reshape(C, 64, 64) for r in res.results])
    return out.astype(np.float32)


# revision 8
# speedup vs baseline: 1.4323x; 1.0018x over previous
"""Cross-attention kernel for Trainium2 (8 NeuronCores, batch-parallel).

Math per batch b (reference semantics):
  q = queries[b].reshape(C, N).T + q_pos        # [N, C]
  k = keys[b].reshape(C, N).T + k_pos
  v = values[b].reshape(C, N).T                 # [N, C]
  out = softmax(q @ k.T / 16) @ v               # [N, Cv]

Device layout (per core = one batch):
  S is computed transposed (S^T[k, q]) so exp(S^T) tiles are directly the
  STATIONARY operand of the O matmul (O[q, c] = sum_k A^T[k, q]^T V[k, c]).
  V chunks are augmented with two ones columns (f32r matmuls need an even
  moving free dim) so the softmax denominator accumulates in PSUM columns
  C/C+1 for free; the final normalization is a per-partition reciprocal +
  scalar multiply on the (otherwise idle) vector engine, staggered into the
  next block's key loop so it never clogs the activation queue.

  Q/K have the position embeddings folded in host-side and are split into
  fp8e4m3 hi+lo halves; S = Kh Qh + Kh Ql + Kl Qh runs as 256-deep
  DoubleRow fp8 matmuls (0.5 PE cycles/row -- 2x the f32r rate).  The
  dropped Kl Ql term and residual quantization contribute ~1e-3 relative
  error on the logits.  The O matmul stays f32r (A stationary, V moving).
"""

import numpy as np

import concourse.bass as bass
import concourse.tile as tile
import concourse.mybir as mybir
from concourse import bacc
from concourse.bass_utils import run_bass_kernel_spmd

P = 128          # partitions
C = 256          # qk/v channel dim
CA = C + 2       # v width augmented with ones columns (must be even)
N = 4096         # sequence (64*64)
B = 8            # batch == n_cores
QW = 512         # query block width (max matmul moving free dim)
NQB = N // QW    # 8 query blocks
NKO = N // P     # 32 key chunks
KPB = QW // P    # key chunks per K block tile
VB = 4           # v chunks loaded per DMA
LAG = 6          # O-matmul lag behind exp, in key chunks
SCALE = 1.0 / 16.0  # 1/sqrt(C)

F32 = mybir.dt.float32
F32R = mybir.dt.float32r
F8 = mybir.dt.float8e4
AF = mybir.ActivationFunctionType
DR = mybir.MatmulPerfMode.DoubleRow

_NC_CACHE = None


def tf32_round(x: np.ndarray) -> np.ndarray:
    u = x.view(np.uint32)
    u = (u + np.uint32(0x1000)) & np.uint32(0xFFFFE000)
    return u.view(np.float32)


def build_nc(ps_s_bufs=4, po_bufs=4, atp_bufs=LAG + 4):
    nc = bacc.Bacc(None, target_bir_lowering=False)
    q8 = nc.dram_tensor("q8", [2, C, N], F8, kind="ExternalInput")
    k8 = nc.dram_tensor("k8", [2, C, N], F8, kind="ExternalInput")
    v = nc.dram_tensor("v", [N, C], F32R, kind="ExternalInput")
    o = nc.dram_tensor("o", [N, C], F32, kind="ExternalOutput")

    q84 = q8.rearrange("hl (co p) n -> p hl co n", p=P)
    k84 = k8.rearrange("hl (co p) n -> p hl co n", p=P)
    v3 = v.rearrange("(g p) c -> p g c", p=P)
    o3 = o.rearrange("(nb p) c -> p nb c", p=P)

    with tile.TileContext(nc) as tc:
        with (
            tc.tile_pool(name="consts", bufs=1) as consts,
            tc.tile_pool(name="kk", bufs=NQB) as kk,
            tc.tile_pool(name="qq", bufs=2) as qq,
            tc.tile_pool(name="vp", bufs=NKO // VB) as vp,
            tc.tile_pool(name="atp", bufs=atp_bufs) as atp,
            tc.tile_pool(name="small", bufs=8) as small,
            tc.tile_pool(name="outp", bufs=2) as outp,
            tc.tile_pool(name="ps_s", bufs=ps_s_bufs, space="PSUM") as ps_s,
            tc.tile_pool(name="ps_o", bufs=po_bufs, space="PSUM") as ps_o,
        ):
            ones_f = consts.tile([P, VB, CA - C], F32, tag="ones_f")
            nc.vector.memset(ones_f, 1.0)
            warm = consts.tile([P, QW], F32R, tag="warm")
            nc.vector.memset(warm, 1.0)

            def load_kblk(j, split=False):
                sl = slice(j * QW, (j + 1) * QW)
                kb = kk.tile([P, 2, 2, QW], F8, tag="k8")
                if split:
                    nc.sync.dma_start(kb[:, :, :, 0:P], k84[:, :, :, 0:P])
                    nc.sync.dma_start(kb[:, :, :, P:QW],
                                      k84[:, :, :, j * QW + P : (j + 1) * QW])
                else:
                    nc.sync.dma_start(kb, k84[:, :, :, sl])
                return kb

            def load_vgroup(g):
                vg = vp.tile([P, VB, CA], F32R, tag="v")
                nc.sync.dma_start(vg[:, :, 0:C], v3[:, g * VB : (g + 1) * VB, :])
                nc.vector.tensor_copy(vg[:, :, C:CA], ones_f)
                return vg

            kblks = {}
            vgs = {}

            def epilogue_piece(j, po, ob, qs, on_act=False, eager_dma=False):
                inv = small.tile([P, 1], F32, tag="inv")
                nc.vector.reciprocal(inv, po[qs][:, C : C + 1])
                if on_act:
                    nc.scalar.activation(ob[:, qs, :], po[qs][:, 0:C],
                                         AF.Copy, scale=inv)
                else:
                    nc.vector.tensor_scalar_mul(ob[:, qs, :], po[qs][:, 0:C],
                                                inv)
                if eager_dma:
                    nc.sync.dma_start(o3[:, 4 * j + qs, :], ob[:, qs, :])
                elif qs == 3:
                    nc.sync.dma_start(o3[:, 4 * j : 4 * j + 4, :], ob)

            pending = None
            for j in range(NQB):
                sl = slice(j * QW, (j + 1) * QW)
                qb = qq.tile([P, 2, 2, QW], F8, tag="q8")
                if j == 0:
                    nc.sync.dma_start(qb[:, 0, :, :], q84[:, 0, :, sl])
                    kblks[0] = load_kblk(0, split=True)
                    nc.sync.dma_start(qb[:, 1, :, :], q84[:, 1, :, sl])
                    # p-state warm-up: keep PE busy while the first loads fly
                    wps = ps_s.tile([P, QW], F32, tag="s", name="wps")
                    for w in range(6):
                        nc.tensor.matmul(wps, warm[:, 0:P], warm,
                                         start=True, stop=True)
                else:
                    nc.sync.dma_start(qb, q84[:, :, :, sl])

                if j == 0:
                    # deadline-ordered remaining loads: K(jb+1) then V(jb)
                    for jb in range(NQB):
                        if jb + 1 < NQB:
                            kblks[jb + 1] = load_kblk(jb + 1)
                        vgs[jb] = load_vgroup(jb)

                po = [ps_o.tile([P, CA], F32, tag="po", name=f"po{qs}",
                                padded_shape=[P, QW]) for qs in range(4)]

                a_q = {}

                def o_matmuls(ko):
                    av = a_q[ko]
                    vc = vgs[ko // VB][:, ko % VB, :]
                    for qs in range(4):
                        nc.tensor.matmul(
                            po[qs],
                            av[:, qs * P : (qs + 1) * P],
                            vc,
                            start=(ko == 0),
                            stop=(ko == NKO - 1),
                        )
                    del a_q[ko]

                for ko in range(NKO):
                    pss = ps_s.tile([P, QW], F32, tag="s")
                    jb, koff = divmod(ko, KPB)
                    ksl = slice(koff * P, (koff + 1) * P)
                    kb = kblks[jb]
                    nc.tensor.matmul(pss, kb[:, 0, :, ksl], qb[:, 0, :, :],
                                     start=True, stop=False, perf_mode=DR)
                    nc.tensor.matmul(pss, kb[:, 0, :, ksl], qb[:, 1, :, :],
                                     start=False, stop=False, perf_mode=DR)
                    nc.tensor.matmul(pss, kb[:, 1, :, ksl], qb[:, 0, :, :],
                                     start=False, stop=True, perf_mode=DR)
                    a = atp.tile([P, QW], F32R, tag="a")
                    nc.scalar.activation(a, pss, AF.Exp, scale=SCALE)
                    a_q[ko] = a

                    if pending is not None and ko < 4:
                        if ko == 0:
                            ob = outp.tile([P, 4, C], F32, tag="ot")
                            pending = (*pending, ob)
                        epilogue_piece(pending[0], pending[1], pending[2], ko)
                        if ko == 3:
                            pending = None

                    if ko >= LAG:
                        o_matmuls(ko - LAG)

                for ko in range(NKO - LAG, NKO):
                    o_matmuls(ko)
                pending = (j, po)

            # final block: split the epilogue across DVE and ACT
            ob = outp.tile([P, 4, C], F32, tag="ot")
            for qs in range(4):
                epilogue_piece(pending[0], pending[1], ob, qs,
                               on_act=(qs % 2 == 1), eager_dma=True)

    nc.compile()
    return nc


def _get_nc():
    global _NC_CACHE
    if _NC_CACHE is None:
        _NC_CACHE = build_nc()
    return _NC_CACHE


def make_in_maps(queries, keys, values, q_pos_embedding, k_pos_embedding):
    queries = np.asarray(queries, dtype=np.float32)
    keys = np.asarray(keys, dtype=np.float32)
    values = np.asarray(values, dtype=np.float32)
    fp8 = mybir.dt.np(F8)
    qpT = np.asarray(q_pos_embedding, dtype=np.float32).reshape(N, C).T
    kpT = np.asarray(k_pos_embedding, dtype=np.float32).reshape(N, C).T
    in_maps = []
    for b in range(B):
        qt = queries[b].reshape(C, N) + qpT
        kt = keys[b].reshape(C, N) + kpT
        qh8 = qt.astype(fp8)
        ql8 = (qt - qh8.astype(np.float32)).astype(fp8)
        kh8 = kt.astype(fp8)
        kl8 = (kt - kh8.astype(np.float32)).astype(fp8)
        vT = tf32_round(np.ascontiguousarray(values[b].reshape(C, N).T))
        in_maps.append({
            "q8": np.ascontiguousarray(np.stack([qh8, ql8])),
            "k8": np.ascontiguousarray(np.stack([kh8, kl8])),
            "v": vT,
        })
    return in_maps


def kernel(queries, keys, values, q_pos_embedding, k_pos_embedding):
    nc = _get_nc()
    in_maps = make_in_maps(queries, keys, values, q_pos_embedding,
                           k_pos_embedding)
    res = run_bass_kernel_spmd(nc, in_maps, core_ids=list(range(B)))
    out = np.stack([r["o"].T.reshape(C, 64, 64) for r in res.results])
    return out.astype(np.float32)


# revision 9
# speedup vs baseline: 1.4357x; 1.0024x over previous
"""Cross-attention kernel for Trainium2 (8 NeuronCores, batch-parallel).

Math per batch b (reference semantics):
  q = queries[b].reshape(C, N).T + q_pos        # [N, C]
  k = keys[b].reshape(C, N).T + k_pos
  v = values[b].reshape(C, N).T                 # [N, C]
  out = softmax(q @ k.T / 16) @ v               # [N, Cv]

Device layout (per core = one batch):
  S is computed transposed (S^T[k, q]) so exp(S^T) tiles are directly the
  STATIONARY operand of the O matmul (O[q, c] = sum_k A^T[k, q]^T V[k, c]).
  V chunks are augmented with two ones columns (f32r matmuls need an even
  moving free dim) so the softmax denominator accumulates in PSUM columns
  C/C+1 for free; the final normalization is a per-partition reciprocal +
  scalar multiply on the (otherwise idle) vector engine, staggered into the
  next block's key loop so it never clogs the activation queue.

  Q/K have the position embeddings folded in host-side and are split into
  fp8e4m3 hi+lo halves; S = Kh Qh + Kh Ql + Kl Qh runs as 256-deep
  DoubleRow fp8 matmuls (0.5 PE cycles/row -- 2x the f32r rate).  The
  dropped Kl Ql term and residual quantization contribute ~1e-3 relative
  error on the logits.  The O matmul stays f32r (A stationary, V moving).
"""

import numpy as np

import concourse.bass as bass
import concourse.tile as tile
import concourse.mybir as mybir
from concourse import bacc
from concourse.bass_utils import run_bass_kernel_spmd

P = 128          # partitions
C = 256          # qk/v channel dim
CA = C + 2       # v width augmented with ones columns (must be even)
N = 4096         # sequence (64*64)
B = 8            # batch == n_cores
QW = 512         # query block width (max matmul moving free dim)
NQB = N // QW    # 8 query blocks
NKO = N // P     # 32 key chunks
KPB = QW // P    # key chunks per K block tile
VB = 4           # v chunks loaded per DMA
LAG = 6          # O-matmul lag behind exp, in key chunks
SCALE = 1.0 / 16.0  # 1/sqrt(C)

F32 = mybir.dt.float32
F32R = mybir.dt.float32r
F8 = mybir.dt.float8e4
AF = mybir.ActivationFunctionType
DR = mybir.MatmulPerfMode.DoubleRow

_NC_CACHE = None


def tf32_round(x: np.ndarray) -> np.ndarray:
    u = x.view(np.uint32)
    u = (u + np.uint32(0x1000)) & np.uint32(0xFFFFE000)
    return u.view(np.float32)


def build_nc(ps_s_bufs=4, po_bufs=4, atp_bufs=LAG + 4):
    nc = bacc.Bacc(None, target_bir_lowering=False)
    q8 = nc.dram_tensor("q8", [2, C, N], F8, kind="ExternalInput")
    k8 = nc.dram_tensor("k8", [2, C, N], F8, kind="ExternalInput")
    v = nc.dram_tensor("v", [N, C], F32R, kind="ExternalInput")
    o = nc.dram_tensor("o", [N, C], F32, kind="ExternalOutput")

    q84 = q8.rearrange("hl (co p) n -> p hl co n", p=P)
    k84 = k8.rearrange("hl (co p) n -> p hl co n", p=P)
    v3 = v.rearrange("(g p) c -> p g c", p=P)
    o3 = o.rearrange("(nb p) c -> p nb c", p=P)

    with tile.TileContext(nc) as tc:
        with (
            tc.tile_pool(name="consts", bufs=1) as consts,
            tc.tile_pool(name="kk", bufs=NQB) as kk,
            tc.tile_pool(name="qq", bufs=2) as qq,
            tc.tile_pool(name="vp", bufs=NKO // VB) as vp,
            tc.tile_pool(name="atp", bufs=atp_bufs) as atp,
            tc.tile_pool(name="small", bufs=8) as small,
            tc.tile_pool(name="outp", bufs=2) as outp,
            tc.tile_pool(name="ps_s", bufs=ps_s_bufs, space="PSUM") as ps_s,
            tc.tile_pool(name="ps_o", bufs=po_bufs, space="PSUM") as ps_o,
        ):
            warm = consts.tile([P, QW], F32R, tag="warm")
            nc.vector.memset(warm, 1.0)
            ones_f = consts.tile([P, VB, CA - C], F32, tag="ones_f")
            nc.vector.memset(ones_f, 1.0)

            def load_kblk(j, split=False):
                sl = slice(j * QW, (j + 1) * QW)
                kb = kk.tile([P, 2, 2, QW], F8, tag="k8")
                if split:
                    nc.sync.dma_start(kb[:, :, :, 0:P], k84[:, :, :, 0:P])
                    nc.sync.dma_start(kb[:, :, :, P:QW],
                                      k84[:, :, :, j * QW + P : (j + 1) * QW])
                else:
                    nc.sync.dma_start(kb, k84[:, :, :, sl])
                return kb

            def load_vgroup(g):
                vg = vp.tile([P, VB, CA], F32R, tag="v")
                nc.sync.dma_start(vg[:, :, 0:C], v3[:, g * VB : (g + 1) * VB, :])
                nc.vector.tensor_copy(vg[:, :, C:CA], ones_f)
                return vg

            kblks = {}
            vgs = {}

            def epilogue_piece(j, po, ob, qs, on_act=False, eager_dma=False):
                inv = small.tile([P, 1], F32, tag="inv")
                nc.vector.reciprocal(inv, po[qs][:, C : C + 1])
                if on_act:
                    nc.scalar.activation(ob[:, qs, :], po[qs][:, 0:C],
                                         AF.Copy, scale=inv)
                else:
                    nc.vector.tensor_scalar_mul(ob[:, qs, :], po[qs][:, 0:C],
                                                inv)
                if eager_dma:
                    nc.sync.dma_start(o3[:, 4 * j + qs, :], ob[:, qs, :])
                elif qs == 3:
                    nc.sync.dma_start(o3[:, 4 * j : 4 * j + 4, :], ob)

            pending = None
            for j in range(NQB):
                sl = slice(j * QW, (j + 1) * QW)
                qb = qq.tile([P, 2, 2, QW], F8, tag="q8")
                if j == 0:
                    nc.sync.dma_start(qb[:, 0, :, :], q84[:, 0, :, sl])
                    kb0 = kk.tile([P, 2, 2, QW], F8, tag="k8", name="kb0")
                    nc.sync.dma_start(kb0[:, :, :, 0:P], k84[:, :, :, 0:P])
                    nc.sync.dma_start(qb[:, 1, :, :], q84[:, 1, :, sl])
                    nc.sync.dma_start(kb0[:, :, :, P:QW], k84[:, :, :, P:QW])
                    kblks[0] = kb0
                    # p-state warm-up: keep PE busy while the first loads fly
                    wps = ps_s.tile([P, QW], F32, tag="s", name="wps")
                    for w in range(4):
                        nc.tensor.matmul(wps, warm[:, 0:P], warm,
                                         start=True, stop=True)
                else:
                    nc.sync.dma_start(qb, q84[:, :, :, sl])

                if j == 0:
                    # deadline-ordered remaining loads: K(jb+1) then V(jb);
                    # the first v group is split so chunk 0 arrives sooner
                    for jb in range(NQB):
                        if jb + 1 < NQB:
                            kblks[jb + 1] = load_kblk(jb + 1)
                        if jb == 0:
                            vg = vp.tile([P, VB, CA], F32R, tag="v", name="vg0")
                            nc.sync.dma_start(vg[:, 0:2, 0:C], v3[:, 0:2, :])
                            nc.sync.dma_start(vg[:, 2:4, 0:C], v3[:, 2:4, :])
                            nc.vector.tensor_copy(vg[:, :, C:CA], ones_f)
                            vgs[0] = vg
                        else:
                            vgs[jb] = load_vgroup(jb)

                po = [ps_o.tile([P, CA], F32, tag="po", name=f"po{qs}",
                                padded_shape=[P, QW]) for qs in range(4)]

                a_q = {}

                def o_matmuls(ko):
                    av = a_q[ko]
                    vc = vgs[ko // VB][:, ko % VB, :]
                    for qs in range(4):
                        nc.tensor.matmul(
                            po[qs],
                            av[:, qs * P : (qs + 1) * P],
                            vc,
                            start=(ko == 0),
                            stop=(ko == NKO - 1),
                        )
                    del a_q[ko]

                for ko in range(NKO):
                    pss = ps_s.tile([P, QW], F32, tag="s")
                    jb, koff = divmod(ko, KPB)
                    ksl = slice(koff * P, (koff + 1) * P)
                    kb = kblks[jb]
                    nc.tensor.matmul(pss, kb[:, 0, :, ksl], qb[:, 0, :, :],
                                     start=True, stop=False, perf_mode=DR)
                    nc.tensor.matmul(pss, kb[:, 0, :, ksl], qb[:, 1, :, :],
                                     start=False, stop=False, perf_mode=DR)
                    nc.tensor.matmul(pss, kb[:, 1, :, ksl], qb[:, 0, :, :],
                                     start=False, stop=True, perf_mode=DR)
                    a = atp.tile([P, QW], F32R, tag="a")
                    nc.scalar.activation(a, pss, AF.Exp, scale=SCALE)
                    a_q[ko] = a

                    if pending is not None and ko < 4:
                        if ko == 0:
                            ob = outp.tile([P, 4, C], F32, tag="ot")
                            pending = (*pending, ob)
                        epilogue_piece(pending[0], pending[1], pending[2], ko)
                        if ko == 3:
                            pending = None

                    if ko >= LAG:
                        o_matmuls(ko - LAG)

                for ko in range(NKO - LAG, NKO):
                    o_matmuls(ko)
                pending = (j, po)

            # final block: split the epilogue across DVE and ACT
            ob = outp.tile([P, 4, C], F32, tag="ot")
            for qs in range(4):
                epilogue_piece(pending[0], pending[1], ob, qs,
                               on_act=(qs % 2 == 1), eager_dma=True)

    nc.compile()
    return nc


def _get_nc():
    global _NC_CACHE
    if _NC_CACHE is None:
        _NC_CACHE = build_nc()
    return _NC_CACHE


def make_in_maps(queries, keys, values, q_pos_embedding, k_pos_embedding):
    queries = np.asarray(queries, dtype=np.float32)
    keys = np.asarray(keys, dtype=np.float32)
    values = np.asarray(values, dtype=np.float32)
    fp8 = mybir.dt.np(F8)
    qpT = np.asarray(q_pos_embedding, dtype=np.float32).reshape(N, C).T
    kpT = np.asarray(k_pos_embedding, dtype=np.float32).reshape(N, C).T
    in_maps = []
    for b in range(B):
        qt = queries[b].reshape(C, N) + qpT
        kt = keys[b].reshape(C, N) + kpT
        qh8 = qt.astype(fp8)
        ql8 = (qt - qh8.astype(np.float32)).astype(fp8)
        kh8 = kt.astype(fp8)
        kl8 = (kt - kh8.astype(np.float32)).astype(fp8)
        vT = tf32_round(np.ascontiguousarray(values[b].reshape(C, N).T))
        in_maps.append({
            "q8": np.ascontiguousarray(np.stack([qh8, ql8])),
            "k8": np.ascontiguousarray(np.stack([kh8, kl8])),
            "v": vT,
        })
    return in_maps


def kernel(queries, keys, values, q_pos_embedding, k_pos_embedding):
    nc = _get_nc()
    in_maps = make_in_maps(queries, keys, values, q_pos_embedding,
                           k_pos_embedding)
    res = run_bass_kernel_spmd(nc, in_maps, core_ids=list(range(B)))
    out = np.stack([r["o"].T.reshape(C, 64, 64) for r in res.results])
    return out.astype(np.float32)


# revision 11
# speedup vs baseline: 1.4451x; 1.0065x over previous
"""Cross-attention kernel for Trainium2 (8 NeuronCores, batch-parallel).

Math per batch b (reference semantics):
  q = queries[b].reshape(C, N).T + q_pos        # [N, C]
  k = keys[b].reshape(C, N).T + k_pos
  v = values[b].reshape(C, N).T                 # [N, C]
  out = softmax(q @ k.T / 16) @ v               # [N, Cv]

Device layout (per core = one batch):
  S is computed transposed (S^T[k, q]) so exp(S^T) tiles are directly the
  STATIONARY operand of the O matmul (O[q, c] = sum_k A^T[k, q]^T V[k, c]).
  V chunks are augmented with two ones columns (f32r matmuls need an even
  moving free dim) so the softmax denominator accumulates in PSUM columns
  C/C+1 for free; the final normalization is a per-partition reciprocal +
  scalar multiply on the (otherwise idle) vector engine, staggered into the
  next block's key loop so it never clogs the activation queue.

  Q/K have the position embeddings folded in host-side and are split into
  fp8e4m3 hi+lo halves; S = Kh Qh + Kh Ql + Kl Qh runs as 256-deep
  DoubleRow fp8 matmuls (0.5 PE cycles/row -- 2x the f32r rate).  The
  dropped Kl Ql term and residual quantization contribute ~1e-3 relative
  error on the logits.  The O matmul stays f32r (A stationary, V moving).
"""

import numpy as np

import concourse.bass as bass
import concourse.tile as tile
import concourse.mybir as mybir
from concourse import bacc
from concourse.bass_utils import run_bass_kernel_spmd

P = 128          # partitions
C = 256          # qk/v channel dim
CA = C + 2       # v width augmented with ones columns (must be even)
N = 4096         # sequence (64*64)
B = 8            # batch == n_cores
QW = 512         # query block width (max matmul moving free dim)
NQB = N // QW    # 8 query blocks
NKO = N // P     # 32 key chunks
KPB = QW // P    # key chunks per K block tile
VB = 4           # v chunks loaded per DMA
LAG = 6          # O-matmul lag behind exp, in key chunks
SCALE = 1.0 / 16.0  # 1/sqrt(C)

F32 = mybir.dt.float32
F32R = mybir.dt.float32r
F8 = mybir.dt.float8e4
AF = mybir.ActivationFunctionType
DR = mybir.MatmulPerfMode.DoubleRow

_NC_CACHE = None


def tf32_round(x: np.ndarray) -> np.ndarray:
    u = x.view(np.uint32)
    u = (u + np.uint32(0x1000)) & np.uint32(0xFFFFE000)
    return u.view(np.float32)


def build_nc(ps_s_bufs=4, po_bufs=4, lag=LAG, n_warm=4, atp_bufs=None):
    atp_bufs = (lag + 4) if atp_bufs is None else atp_bufs
    nc = bacc.Bacc(None, target_bir_lowering=False)
    q8 = nc.dram_tensor("q8", [2, C, N], F8, kind="ExternalInput")
    k8 = nc.dram_tensor("k8", [2, C, N], F8, kind="ExternalInput")
    v = nc.dram_tensor("v", [N, C], F32R, kind="ExternalInput")
    o = nc.dram_tensor("o", [N, C], F32, kind="ExternalOutput")

    q84 = q8.rearrange("hl (co p) n -> p hl co n", p=P)
    k84 = k8.rearrange("hl (co p) n -> p hl co n", p=P)
    v3 = v.rearrange("(g p) c -> p g c", p=P)
    o3 = o.rearrange("(nb p) c -> p nb c", p=P)

    with tile.TileContext(nc) as tc:
        with (
            tc.tile_pool(name="consts", bufs=1) as consts,
            tc.tile_pool(name="kk", bufs=NQB) as kk,
            tc.tile_pool(name="qq", bufs=2) as qq,
            tc.tile_pool(name="vp", bufs=NKO // VB) as vp,
            tc.tile_pool(name="atp", bufs=atp_bufs) as atp,
            tc.tile_pool(name="small", bufs=8) as small,
            tc.tile_pool(name="outp", bufs=2) as outp,
            tc.tile_pool(name="ps_s", bufs=ps_s_bufs, space="PSUM") as ps_s,
            tc.tile_pool(name="ps_o", bufs=po_bufs, space="PSUM") as ps_o,
        ):
            warm = consts.tile([P, QW], F32R, tag="warm")
            nc.vector.memset(warm, 1.0)
            ones_f = consts.tile([P, VB, CA - C], F32, tag="ones_f")
            nc.vector.memset(ones_f, 1.0)

            def load_kblk(j, split=False):
                sl = slice(j * QW, (j + 1) * QW)
                kb = kk.tile([P, 2, 2, QW], F8, tag="k8")
                if split:
                    nc.sync.dma_start(kb[:, :, :, 0:P], k84[:, :, :, 0:P])
                    nc.sync.dma_start(kb[:, :, :, P:QW],
                                      k84[:, :, :, j * QW + P : (j + 1) * QW])
                else:
                    nc.sync.dma_start(kb, k84[:, :, :, sl])
                return kb

            def load_vgroup(g):
                vg = vp.tile([P, VB, CA], F32R, tag="v")
                nc.sync.dma_start(vg[:, :, 0:C], v3[:, g * VB : (g + 1) * VB, :])
                nc.vector.tensor_copy(vg[:, :, C:CA], ones_f)
                return vg

            kblks = {}
            vgs = {}

            def epilogue_piece(j, po, ob, qs, on_act=False, eager_dma=False):
                inv = small.tile([P, 1], F32, tag="inv")
                nc.vector.reciprocal(inv, po[qs][:, C : C + 1])
                if on_act:
                    nc.scalar.activation(ob[:, qs, :], po[qs][:, 0:C],
                                         AF.Copy, scale=inv)
                else:
                    nc.vector.tensor_scalar_mul(ob[:, qs, :], po[qs][:, 0:C],
                                                inv)
                if eager_dma:
                    nc.sync.dma_start(o3[:, 4 * j + qs, :], ob[:, qs, :])
                elif qs == 3:
                    nc.sync.dma_start(o3[:, 4 * j : 4 * j + 4, :], ob)

            pending = None
            for j in range(NQB):
                sl = slice(j * QW, (j + 1) * QW)
                qb = qq.tile([P, 2, 2, QW], F8, tag="q8")
                if j == 0:
                    nc.sync.dma_start(qb[:, 0, :, :], q84[:, 0, :, sl])
                    kb0 = kk.tile([P, 2, 2, QW], F8, tag="k8", name="kb0")
                    nc.sync.dma_start(kb0[:, :, :, 0:P], k84[:, :, :, 0:P])
                    nc.sync.dma_start(qb[:, 1, :, :], q84[:, 1, :, sl])
                    nc.sync.dma_start(kb0[:, :, :, P:QW], k84[:, :, :, P:QW])
                    kblks[0] = kb0
                    # p-state warm-up: keep PE busy while the first loads fly
                    wps = ps_s.tile([P, QW], F32, tag="s", name="wps")
                    for w in range(n_warm):
                        nc.tensor.matmul(wps, warm[:, 0:P], warm,
                                         start=True, stop=True)
                else:
                    nc.sync.dma_start(qb, q84[:, :, :, sl])

                if j == 0:
                    # deadline-ordered remaining loads: K(jb+1) then V(jb);
                    # the first v group is split so chunk 0 arrives sooner
                    for jb in range(NQB):
                        if jb + 1 < NQB:
                            kblks[jb + 1] = load_kblk(jb + 1)
                        if jb == 0:
                            vg = vp.tile([P, VB, CA], F32R, tag="v", name="vg0")
                            nc.sync.dma_start(vg[:, 0:2, 0:C], v3[:, 0:2, :])
                            nc.sync.dma_start(vg[:, 2:4, 0:C], v3[:, 2:4, :])
                            nc.vector.tensor_copy(vg[:, :, C:CA], ones_f)
                            vgs[0] = vg
                        else:
                            vgs[jb] = load_vgroup(jb)

                po = [ps_o.tile([P, CA], F32, tag="po", name=f"po{qs}",
                                padded_shape=[P, QW]) for qs in range(4)]

                a_q = {}

                def o_matmuls(ko):
                    av = a_q[ko]
                    vc = vgs[ko // VB][:, ko % VB, :]
                    for qs in range(4):
                        nc.tensor.matmul(
                            po[qs],
                            av[:, qs * P : (qs + 1) * P],
                            vc,
                            start=(ko == 0),
                            stop=(ko == NKO - 1),
                        )
                    del a_q[ko]

                for ko in range(NKO):
                    pss = ps_s.tile([P, QW], F32, tag="s")
                    jb, koff = divmod(ko, KPB)
                    ksl = slice(koff * P, (koff + 1) * P)
                    kb = kblks[jb]
                    nc.tensor.matmul(pss, kb[:, 0, :, ksl], qb[:, 0, :, :],
                                     start=True, stop=False, perf_mode=DR)
                    nc.tensor.matmul(pss, kb[:, 0, :, ksl], qb[:, 1, :, :],
                                     start=False, stop=False, perf_mode=DR)
                    nc.tensor.matmul(pss, kb[:, 1, :, ksl], qb[:, 0, :, :],
                                     start=False, stop=True, perf_mode=DR)
                    a = atp.tile([P, QW], F32R, tag="a")
                    nc.scalar.activation(a, pss, AF.Exp, scale=SCALE)
                    a_q[ko] = a

                    if pending is not None and ko < 4:
                        if ko == 0:
                            ob = outp.tile([P, 4, C], F32, tag="ot")
                            pending = (*pending, ob)
                        epilogue_piece(pending[0], pending[1], pending[2], ko)
                        if ko == 3:
                            pending = None

                    if ko >= lag:
                        o_matmuls(ko - lag)

                if j < NQB - 1:
                    for ko in range(NKO - lag, NKO):
                        o_matmuls(ko)
                    pending = (j, po)
                else:
                    # final block: drain qs-major so each accumulation group
                    # closes early and its epilogue+store pipelines behind
                    # the remaining matmuls
                    ob = outp.tile([P, 4, C], F32, tag="ot")
                    for qs in range(4):
                        for ko in range(NKO - lag, NKO):
                            nc.tensor.matmul(
                                po[qs],
                                a_q[ko][:, qs * P : (qs + 1) * P],
                                vgs[ko // VB][:, ko % VB, :],
                                start=False,
                                stop=(ko == NKO - 1),
                            )
                        epilogue_piece(j, po, ob, qs, on_act=(qs % 2 == 1),
                                       eager_dma=True)

    nc.compile()
    return nc


def _get_nc():
    global _NC_CACHE
    if _NC_CACHE is None:
        _NC_CACHE = build_nc()
    return _NC_CACHE


def make_in_maps(queries, keys, values, q_pos_embedding, k_pos_embedding):
    queries = np.asarray(queries, dtype=np.float32)
    keys = np.asarray(keys, dtype=np.float32)
    values = np.asarray(values, dtype=np.float32)
    fp8 = mybir.dt.np(F8)
    qpT = np.asarray(q_pos_embedding, dtype=np.float32).reshape(N, C).T
    kpT = np.asarray(k_pos_embedding, dtype=np.float32).reshape(N, C).T
    in_maps = []
    for b in range(B):
        qt = queries[b].reshape(C, N) + qpT
        kt = keys[b].reshape(C, N) + kpT
        qh8 = qt.astype(fp8)
        ql8 = (qt - qh8.astype(np.float32)).astype(fp8)
        kh8 = kt.astype(fp8)
        kl8 = (kt - kh8.astype(np.float32)).astype(fp8)
        vT = tf32_round(np.ascontiguousarray(values[b].reshape(C, N).T))
        in_maps.append({
            "q8": np.ascontiguousarray(np.stack([qh8, ql8])),
            "k8": np.ascontiguousarray(np.stack([kh8, kl8])),
            "v": vT,
        })
    return in_maps


def kernel(queries, keys, values, q_pos_embedding, k_pos_embedding):
    nc = _get_nc()
    in_maps = make_in_maps(queries, keys, values, q_pos_embedding,
                           k_pos_embedding)
    res = run_bass_kernel_spmd(nc, in_maps, core_ids=list(range(B)))
    out = np.stack([r["o"].T.reshape(C, 64, 64) for r in res.results])
    return out.astype(np.float32)
